# revision 39
# baseline (speedup 1.0000x reference)
"""Trainium2 Bass kernel for nn_MemoryModule3D (scatter_memory).

Computes, for z (8,256,8,16,16) and memory (2000,256):
  cosine-similarity logits -> softmax over memory slots -> hard-shrink
  (lambda=1/2000) -> L1 renorm -> readout.
Returns (z_hat (8,256,8,16,16) f32, w_hat (8,2000,2048) f32), matching the
reference's two outputs.

Sharding: data-parallel over N=8 batch elements, one per NeuronCore, with
the 2000x256 memory table replicated (forward only -> no collectives).

Per-core layout: [L, M] (spatial l on partitions, memory slot m on free dim).
Key algebraic simplification: the final L1 renorm makes the softmax
denominator cancel everywhere except inside the shrink comparison, so
  w_hat = g / sum_m(g),  g = e * 1[e > lam * s],  e = exp(logit), s = sum_m e
No division by s, no max-subtraction (cosine logits are bounded in [-1,1]).

Precision plan (probed on HW):
  - mm1 logits: bf16 hi/lo 3-pass (hi*hi + hi*lo + lo*hi) -> ~2^-18 operand
    precision at 1 cyc/row; the shrink threshold comparison needs fp32-level
    logits (threshold flips dominate w_hat error), values only need ~1e-4.
  - z-column norm folded into ACT Exp's per-partition scale (mm1 runs on
    raw z; memory side is pre-normalized).
  - PE transpose with fp32 identity is bit-exact; strips stored as f32r
    (~1e-4 value rounding) feed both the w_hat DMA and the f32r mm2 readout.

Pipeline per l-tile (16 tiles of 128 l's):
  mm1 (PE, 6 matmuls x 4 m-chunks)      -> psum quarters
  ACT Exp(scale=znr) + accum_out        -> e tile + s
  DVE scalar_tensor_tensor + accum      -> g tile + L1   (shrink mask)
  DVE tensor_scalar_mul by r=1/L1       -> w tile
  PE transpose 16 blocks                -> psum -> f32r strips
  per group of 4 l-tiles: mm2 (f32r) -> z_hat block; DMA strips + z_hat
"""

import numpy as np

import concourse.bacc as bacc
import concourse.mybir as mybir
import concourse.tile as tile
from concourse.masks import make_identity
from concourse.bass_utils import run_bass_kernel_spmd

F32 = mybir.dt.float32
F32R = mybir.dt.float32r
BF16 = mybir.dt.bfloat16
AF = mybir.ActivationFunctionType
ALU = mybir.AluOpType

N, C, D, H, W = 8, 256, 8, 16, 16
L = D * H * W            # 2048
M = 2000
LAM = 1.0 / M
NCORES = 8

NLT = L // 128           # 16 l-tiles
NMB = (M + 127) // 128   # 16 m-blocks (15x128 + 80)
GROUP = 4                # l-tiles per mm2/output group
GL = GROUP * 128         # 512 l-cols per group
MCH = ((0, 512), (512, 512), (1024, 512), (1536, 464))  # psum-bank m-chunks


def _mblk(k):
    return min(128, M - k * 128)


def _build():
    nc = bacc.Bacc(None, target_bir_lowering=False)

    z_n = nc.dram_tensor("z_n", [C, L], F32, kind="ExternalInput")
    mem_d = nc.dram_tensor("memory", [M, C], F32, kind="ExternalInput")
    zh_d = nc.dram_tensor("z_hat", [C, L], F32, kind="ExternalOutput")
    wh_d = nc.dram_tensor("w_hat", [M, L], F32, kind="ExternalOutput")

    with tile.TileContext(nc) as tc:
        with (
            tc.tile_pool(name="persist", bufs=1) as pp,
            tc.tile_pool(name="smalls", bufs=24) as sp,
        ):
            # ---------------- persistent tiles ----------------
            zf_hi = [pp.tile([128, L], BF16, tag=f"zfh{c}", name=f"zfh{c}")
                     for c in range(2)]
            zf_lo = [pp.tile([128, L], BF16, tag=f"zfl{c}", name=f"zfl{c}")
                     for c in range(2)]
            mT_hi = [pp.tile([128, M], BF16, tag=f"mTh{c}", name=f"mTh{c}")
                     for c in range(2)]
            mT_lo = [pp.tile([128, M], BF16, tag=f"mTl{c}", name=f"mTl{c}")
                     for c in range(2)]
            mem_r = [pp.tile([_mblk(k), C], F32R, tag=f"memr{k}",
                             name=f"memr{k}") for k in range(NMB)]
            ident = pp.tile([128, 128], F32, tag="ident", name="ident")
            make_identity(nc, ident)
            ident_r = pp.tile([128, 128], F32R, tag="identr", name="ident_r")
            nc.vector.tensor_copy(out=ident_r, in_=ident)
            # znr_cols[:, i] = rsqrt(sum_c z[c, l]^2) for l-tile i
            znr_cols = pp.tile([128, NLT], F32, tag="znr", name="znr_cols")

            # ---------------- setup ----------------
            with (
                tc.tile_pool(name="setup_sb", bufs=1) as st,
                tc.tile_pool(name="setup_ps", bufs=1, space="PSUM") as stp,
            ):
                zf = [st.tile([128, L], F32, tag=f"zf{c}", name=f"zf{c}")
                      for c in range(2)]
                for c in range(2):
                    for hh in range(2):
                        nc.sync.dma_start(
                            out=zf[c][:, hh * 1024:(hh + 1) * 1024],
                            in_=z_n[c * 128:(c + 1) * 128,
                                    hh * 1024:(hh + 1) * 1024])
                for c in range(2):
                    nc.vector.tensor_copy(out=zf_hi[c], in_=zf[c])
                    nc.vector.tensor_tensor(out=zf_lo[c], in0=zf[c],
                                            in1=zf_hi[c], op=ALU.subtract)

                # z col norms: psum_z[:, i] = sum_c zf[c, ltile_i]^2 via
                # per-l-tile matmuls against a ones column (lhsT = zsq slice).
                zsq = [st.tile([128, L], F32, tag=f"zsq{c}", name=f"zsq{c}")
                       for c in range(2)]
                for c in range(2):
                    nc.scalar.activation(out=zsq[c], in_=zf[c], func=AF.Square)
                ones1 = st.tile([128, 1], F32, tag="ones1", name="ones1")
                nc.vector.memset(ones1, 1.0)
                p_z = stp.tile([128, NLT], F32, tag="p_z", name="p_z")
                for i in range(NLT):
                    for c in range(2):
                        nc.tensor.matmul(
                            p_z[:, i:i + 1],
                            lhsT=zsq[c][:, i * 128:(i + 1) * 128],
                            rhs=ones1, start=(c == 0), stop=(c == 1),
                        )
                nc.scalar.activation(out=znr_cols, in_=p_z, func=AF.Sqrt)
                nc.vector.reciprocal(out=znr_cols, in_=znr_cols)

                # memory: row norms, normalize, transpose to memnT, hi/lo
                memnT = [st.tile([128, M], F32, tag=f"mT{c}", name=f"mT{c}")
                         for c in range(2)]
                p_mt = stp.tile([128, 1024], F32, tag="p_mt", name="p_mt")
                memn = [st.tile([_mblk(k), C], F32, tag=f"memn{k}",
                                name=f"memn{k}") for k in range(NMB)]
                for k in range(NMB):
                    pk = _mblk(k)
                    eng = nc.gpsimd
                    eng.dma_start(out=memn[k],
                                  in_=mem_d[k * 128:k * 128 + pk, :])
                with tc.tile_pool(name="mscr_p", bufs=4) as mscrp:
                    for k in range(NMB):
                        pk = _mblk(k)
                        nc.vector.tensor_copy(out=mem_r[k], in_=memn[k])
                        mscr = mscrp.tile([128, C], F32, tag="mscr",
                                          name="mscr")
                        msq = sp.tile([pk, 1], F32, tag="msq", name="msq")
                        nc.scalar.activation(out=mscr[:pk, :], in_=memn[k],
                                             func=AF.Square, accum_out=msq)
                        mn = sp.tile([pk, 1], F32, tag="mn", name="mn")
                        nc.scalar.activation(out=mn, in_=msq, func=AF.Sqrt)
                        rmn = sp.tile([pk, 1], F32, tag="rmn", name="rmn")
                        nc.vector.reciprocal(out=rmn, in_=mn)
                        nc.vector.tensor_scalar_mul(out=memn[k], in0=memn[k],
                                                    scalar1=rmn)
                for c in range(2):
                    for hs, hk in ((0, 8), (8, NMB)):
                        off = 0
                        for k in range(hs, hk):
                            pk = _mblk(k)
                            nc.tensor.transpose(
                                p_mt[:, off:off + pk],
                                memn[k][:, c * 128:(c + 1) * 128],
                                ident[:pk, :pk],
                            )
                            off += pk
                        nc.scalar.copy(
                            out=memnT[c][:, hs * 128:hs * 128 + off],
                            in_=p_mt[:, :off],
                        )
                for c in range(2):
                    nc.vector.tensor_copy(out=mT_hi[c], in_=memnT[c])
                    nc.vector.tensor_tensor(out=mT_lo[c], in0=memnT[c],
                                            in1=mT_hi[c], op=ALU.subtract)

            # ---------------- main pipeline ----------------
            with (
                tc.tile_pool(name="ps_a", bufs=5, space="PSUM") as psa,
                tc.tile_pool(name="ps_t", bufs=2, space="PSUM") as pst,
                tc.tile_pool(name="ps_m", bufs=1, space="PSUM") as psm,
                tc.tile_pool(name="e_p", bufs=3) as ep,
                tc.tile_pool(name="g_p", bufs=3) as gp,
                tc.tile_pool(name="w_p", bufs=3) as wp,
                tc.tile_pool(name="strip", bufs=2) as strp,
                tc.tile_pool(name="zh_p", bufs=2) as zhp,
            ):
                GROUPS = (2, 3, 4, 4, 3)  # l-tiles per group (sum = NLT)
                gmeta = []  # per tile: (group_idx, within-group idx, gl0, glen)
                gl0 = 0
                for gi, gn in enumerate(GROUPS):
                    for j in range(gn):
                        gmeta.append((gi, j, gl0, gn))
                    gl0 += gn

                # PE executes its stream in order, so the transpose/mm2
                # consumer stage of tile i is emitted after tile i+1's mm1 --
                # by the time PE reaches it, the ACT/DVE chain for tile i has
                # finished and PE never stalls on it.
                state = {"wt_g": None}

                def producer(i):
                    """mm1 + exp + shrink mask + scale -> w tile (f32r)."""
                    e_t = ep.tile([128, M], F32, tag="e", name="e_t")
                    lsl = slice(i * 128, (i + 1) * 128)
                    terms = []
                    for c in range(2):
                        terms += [(zf_hi[c], mT_hi[c]), (zf_hi[c], mT_lo[c]),
                                  (zf_lo[c], mT_hi[c])]
                    s_h = []
                    for mo, mw in MCH:
                        pa = psa.tile([128, 512], F32, tag="pa", name="pa")
                        for ci, (lhs, rhs) in enumerate(terms):
                            nc.tensor.matmul(
                                pa[:, :mw], lhsT=lhs[:, lsl],
                                rhs=rhs[:, mo:mo + mw],
                                start=(ci == 0), stop=(ci == len(terms) - 1),
                            )
                        sh = sp.tile([128, 1], F32, tag="sh", name="sh")
                        nc.scalar.activation(
                            out=e_t[:, mo:mo + mw], in_=pa[:, :mw],
                            func=AF.Exp, scale=znr_cols[:, i:i + 1],
                            accum_out=sh,
                        )
                        s_h.append(sh)

                    t_t = sp.tile([128, 1], F32, tag="t", name="t_t")
                    nc.vector.tensor_add(out=t_t, in0=s_h[0], in1=s_h[1])
                    for sh_ in s_h[2:]:
                        nc.vector.tensor_add(out=t_t, in0=t_t, in1=sh_)
                    nc.vector.tensor_scalar_mul(out=t_t, in0=t_t, scalar1=LAM)

                    g_t = gp.tile([128, M], F32, tag="g", name="g_t")
                    l1 = sp.tile([128, 1], F32, tag="l1", name="l1")
                    nc.vector.scalar_tensor_tensor(
                        out=g_t, in0=e_t, scalar=t_t, in1=e_t,
                        op0=ALU.is_gt, op1=ALU.mult, accum_out=l1,
                    )
                    rr = sp.tile([128, 1], F32, tag="rr", name="rr")
                    nc.vector.tensor_scalar_max(out=rr, in0=l1, scalar1=1e-30)
                    nc.vector.reciprocal(out=rr, in_=rr)

                    w_t = wp.tile([128, 2048], F32R, tag="w", name="w_t")
                    nc.vector.tensor_scalar_mul(out=w_t[:, :M], in0=g_t,
                                                scalar1=rr)
                    nc.gpsimd.memset(w_t[:, M:].bitcast(mybir.dt.uint32), 0)
                    return w_t

                def consumer(i, w_t):
                    """transposes -> strips; on group end: DMAs + mm2."""
                    g_idx, li, gtile0, glen = gmeta[i]
                    if li == 0:
                        state["wt_g"] = strp.tile(
                            [128, NMB, glen * 128], F32R, tag="wt", name="wt")
                    wt_g = state["wt_g"]
                    for q in range(4):
                        pt = pst.tile([128, 512], F32, tag="pt", name="pt")
                        for j in range(4):
                            k = q * 4 + j
                            nc.tensor.transpose(
                                pt.bitcast(F32R)[:, j * 128:(j + 1) * 128],
                                w_t[:, k * 128:(k + 1) * 128], ident_r,
                            )
                        if q % 2 == 0:
                            nc.scalar.copy(
                                out=wt_g[:, q * 4:(q + 1) * 4,
                                         li * 128:(li + 1) * 128],
                                in_=pt.rearrange("p (k l) -> p k l", k=4),
                            )
                        else:
                            nc.vector.tensor_copy(
                                out=wt_g[:, q * 4:(q + 1) * 4,
                                         li * 128:(li + 1) * 128],
                                in_=pt.rearrange("p (k l) -> p k l", k=4),
                            )

                    if li == glen - 1:
                        lc0 = gtile0 * 128
                        lcw = glen * 128
                        for k in range(NMB):
                            pk = _mblk(k)
                            eng = nc.gpsimd if k % 2 == 0 else nc.sync
                            eng.dma_start(
                                out=wh_d[k * 128:k * 128 + pk,
                                         lc0:lc0 + lcw],
                                in_=wt_g[:pk, k, :].bitcast(F32),
                            )
                        for c in range(2):
                            pm = psm.tile([128, 512], F32, tag="pm", name="pm")
                            for k in range(NMB):
                                pk = _mblk(k)
                                nc.tensor.matmul(
                                    pm[:, :lcw],
                                    lhsT=mem_r[k][:, c * 128:(c + 1) * 128],
                                    rhs=wt_g[:pk, k, :],
                                    start=(k == 0), stop=(k == NMB - 1),
                                )
                            zh_t = zhp.tile([128, 512], F32, tag="zh",
                                            name="zh_t")
                            nc.scalar.copy(out=zh_t[:, :lcw], in_=pm[:, :lcw])
                            nc.sync.dma_start(
                                out=zh_d[c * 128:(c + 1) * 128,
                                         lc0:lc0 + lcw],
                                in_=zh_t[:, :lcw],
                            )

                pending = None
                for i in range(NLT):
                    w_t = producer(i)
                    if pending is not None:
                        consumer(*pending)
                    pending = (i, w_t)
                consumer(*pending)

    nc.compile()
    return nc


_NC_CACHE = None
TRACE = False
LAST_RESULTS = None


def kernel(z, memory):
    global _NC_CACHE, LAST_RESULTS
    if _NC_CACHE is None:
        _NC_CACHE = _build()
    nc = _NC_CACHE

    z = np.ascontiguousarray(np.asarray(z, dtype=np.float32))
    memory = np.ascontiguousarray(np.asarray(memory, dtype=np.float32))
    zf = z.reshape(N, C, L)
    in_maps = [{"z_n": np.ascontiguousarray(zf[n]), "memory": memory}
               for n in range(NCORES)]
    res = run_bass_kernel_spmd(nc, in_maps, core_ids=list(range(NCORES)),
                               trace=TRACE)
    LAST_RESULTS = res
    z_hat = np.stack([r["z_hat"] for r in res.results]).reshape(N, C, D, H, W)
    w_hat = np.stack([r["w_hat"] for r in res.results])
    return z_hat, w_hat
